# revision 27
# baseline (speedup 1.0000x reference)
"""Trainium2 Bass kernel: BERT(12L)+CRF loss, data-parallel over batch on 8 cores.

Self-contained: hardcodes shapes from the problem spec. Each core processes 4
sequences end-to-end on device (embeddings, 12 encoder layers, log-softmax
emissions, CRF numerator + log-partition via a log-semiring reduction tree) and
emits per-sequence partial sums; the host sums 8 cores' partials into the
scalar loss.
"""

import sys

sys.path.insert(0, "/opt/trn_rl_repo")

import numpy as np
import ml_dtypes

import concourse.bass as bass
import concourse.bacc as bacc
import concourse.mybir as mybir
import concourse.tile as tile
from concourse.bass_utils import run_bass_kernel_spmd
from concourse.masks import make_identity

B, S, V, H, NL, NH, FF, L = 32, 256, 30522, 768, 12, 12, 3072, 9
DH = H // NH
SCALE = 1.0 / float(np.sqrt(DH))
NCORES = 8
BL = B // NCORES          # 4 sequences per core
T = BL * S                # 1024 tokens per core
P = 128
TT = T // P               # 8 token tiles
FB = H // P               # 6 feature blocks
FFB = FF // P             # 24 ff blocks
NEG = -30.0               # log-semiring pad "-inf"
LNEPS = 1e-12

F32 = mybir.dt.float32
BF16 = mybir.dt.bfloat16
FP8 = mybir.dt.float8e4
I32 = mybir.dt.int32
AF = mybir.ActivationFunctionType
ALU = mybir.AluOpType
AX = mybir.AxisListType
BF16NP = ml_dtypes.bfloat16


def _xap(ap, dims):
    """Raw AP with the partition dim of `ap` and explicit free dims [(step, count)...]."""
    return bass.AP(tensor=ap.tensor, offset=ap.offset,
                   ap=[list(ap.ap[0])] + [[s, n] for (s, n) in dims])


def _steer_act_tables(nc):
    """Steer ACT table-set selection: route Exp and Ln to the combined
    natural_log_exp set so the CRF tail doesn't ping-pong between the
    exp-only and ln-only sets (each switch costs ~2.7us on hardware)."""
    from concourse.hw_specs import get_activation_tables
    try:
        tabs = get_activation_tables(nc.m.arch)
    except Exception:
        return
    both = tabs.get("natural_log_exp_and_others")
    if not both or AF.Exp not in both:
        return
    ex = tabs.get("exp_and_others")
    if ex is not None:
        ex.discard(AF.Exp)
    nl_ = tabs.get("natural_log")
    if nl_ is not None:
        nl_.discard(AF.Ln)


def build(nl=NL, taps=(), use_bias=False, use_lngb=False):
    """Emit the full per-core program. Returns (nc, tap_names)."""
    nc = bacc.Bacc("TRN2", target_bir_lowering=False, debug=False)
    _steer_act_tables(nc)
    dt_in = lambda name, shape, dt: nc.dram_tensor(name, shape, dt, kind="ExternalInput").ap()
    dt_out = lambda name, shape, dt: nc.dram_tensor(name, shape, dt, kind="ExternalOutput").ap()

    tok_d = dt_in("tok", [T, 1], I32)
    typ_d = dt_in("typ", [T, 1], I32)
    y_d = dt_in("y", [T, 1], F32)
    wemb_d = dt_in("wemb", [V, H], BF16)
    pemb_d = dt_in("pemb", [S, H], BF16)
    temb_d = dt_in("temb", [2, H], BF16)
    KB2 = H // 256      # contract pair-chunks over H (DoubleRow fp8)
    FF2 = FF // 256     # contract pair-chunks over FF
    wq_d = dt_in("wq", [nl, KB2, P, 2 * H], FP8)
    wk_d = dt_in("wk", [nl, KB2, P, 2 * H], FP8)
    wv_d = dt_in("wv", [nl, KB2, P, 2 * H], FP8)
    wo_d = dt_in("wo", [nl, KB2, P, 2 * H], FP8)
    w1_d = dt_in("w1", [nl, KB2, P, 2 * FF], FP8)
    w2_d = dt_in("w2", [nl, FF2, P, 2 * H], FP8)
    fcw_d = dt_in("fcw", [H, L], BF16)
    trans_d = dt_in("ctrans", [1, L * L], F32)
    transT_d = dt_in("ctransT", [1, L * L], F32)
    start_d = dt_in("cstart", [1, L], F32)
    end_d = dt_in("cend", [1, L], F32)
    iota9_d = dt_in("iota9", [P, L], F32)
    iota81_d = dt_in("iota81", [P, L * L], F32)
    tmask_d = dt_in("tmask", [T, 1], F32)
    pad81_d = dt_in("pad81", [1, L * L], F32)
    if use_bias:
        bq_d = dt_in("bq", [nl, 1, H], F32)
        bk_d = dt_in("bk", [nl, 1, H], F32)
        bv_d = dt_in("bv", [nl, 1, H], F32)
        bo_d = dt_in("bo", [nl, 1, H], F32)
        b1_d = dt_in("b1", [nl, 1, FF], F32)
        b2_d = dt_in("b2", [nl, 1, H], F32)
    if use_lngb:
        lng_d = dt_in("lng", [2 * nl + 1, 1, H], F32)  # emb, (ln1,ln2)*nl
        lnb_d = dt_in("lnb", [2 * nl + 1, 1, H], F32)

    nums_d = dt_out("nums", [1, TT], F32)
    dens_d = dt_out("dens", [1, BL], F32)
    tap_d = {}
    for tp in taps:
        if tp == "em":
            tap_d[tp] = dt_out("tap_em", [T, L], F32)
        else:
            tap_d[tp] = dt_out("tap_" + tp, [T, H], F32)

    with tile.TileContext(nc) as tc:
        from contextlib import ExitStack
        with ExitStack() as ctx:
            pers = ctx.enter_context(tc.tile_pool(name="pers", bufs=1))
            bigp = ctx.enter_context(tc.tile_pool(name="bigp", bufs=4))
            g1p = ctx.enter_context(tc.tile_pool(name="g1p", bufs=1))
            wpp = ctx.enter_context(tc.tile_pool(name="wpp", bufs=14))
            w1p = ctx.enter_context(tc.tile_pool(name="w1p", bufs=4))
            w2p = ctx.enter_context(tc.tile_pool(name="w2p", bufs=14))
            esp = ctx.enter_context(tc.tile_pool(name="esp", bufs=4))
            smp = ctx.enter_context(tc.tile_pool(name="smp", bufs=4))
            cmp_ = ctx.enter_context(tc.tile_pool(name="cmp", bufs=6))
            ps_mm = ctx.enter_context(tc.tile_pool(name="ps_mm", bufs=4, space="PSUM"))
            ps_st = ctx.enter_context(tc.tile_pool(name="ps_st", bufs=2, space="PSUM"))
            ps_cx = ctx.enter_context(tc.tile_pool(name="ps_cx", bufs=2, space="PSUM"))
            drp = ctx.enter_context(tc.tile_pool(name="drp", bufs=1, space="DRAM"))

            # ---- constants ----
            idbf = pers.tile([P, P], BF16)
            make_identity(nc, idbf[:])
            ones_col = pers.tile([P, 1], F32)
            nc.vector.memset(ones_col[:], 1.0)
            eps_t = pers.tile([P, 1], F32)
            nc.vector.memset(eps_t[:], LNEPS)
            iota9 = pers.tile([P, L], F32)
            nc.sync.dma_start(out=iota9[:], in_=iota9_d[:])
            iota81 = pers.tile([P, L * L], F32)
            nc.sync.dma_start(out=iota81[:], in_=iota81_d[:])
            transf = pers.tile([P, L * L], F32)
            nc.sync.dma_start(out=transf[:], in_=bass.AP(
                tensor=trans_d.tensor, offset=0, ap=[[0, P], [1, L * L]]))
            transfT = pers.tile([P, L * L], F32)
            nc.sync.dma_start(out=transfT[:], in_=bass.AP(
                tensor=transT_d.tensor, offset=0, ap=[[0, P], [1, L * L]]))
            startf = pers.tile([P, L], F32)
            nc.vector.memset(startf[:], 0.0)
            nc.sync.dma_start(out=startf[0:1, :], in_=start_d[:])
            endf = pers.tile([P, L], F32)
            nc.vector.memset(endf[:], 0.0)
            nc.sync.dma_start(out=endf[P - 1:P, :], in_=end_d[:])
            fcw_sb = pers.tile([P, FB, L], BF16)
            nc.sync.dma_start(out=fcw_sb[:], in_=fcw_d[:].rearrange("(kb p) l -> p kb l", p=P))

            h_res = pers.tile([P, TT, H], BF16)
            em_sb = pers.tile([P, TT, L], F32)
            part_all = pers.tile([P, TT], F32)

            def ln_gb_fields(idx):
                if not use_lngb:
                    return None
                gf = bigp.tile([P, H], F32, tag="big")
                bf = bigp.tile([P, H], F32, tag="big")
                nc.sync.dma_start(out=gf[:], in_=bass.AP(
                    tensor=lng_d.tensor, offset=idx * H, ap=[[0, P], [1, H]]))
                nc.sync.dma_start(out=bf[:], in_=bass.AP(
                    tensor=lnb_d.tensor, offset=idx * H, ap=[[0, P], [1, H]]))
                return gf, bf

            # LayerNorm split into per-tile stats, one batched rsqrt for all 8
            # tiles (2 tiny ACT ops in the natural_log_exp set -> no table
            # switches near Gelu), then per-tile apply.
            def ln_stats(x_ap, mvt, tt):
                stats = smp.tile([P, 3, 6], F32, tag="st9")
                for sg in range(3):
                    nc.vector.bn_stats(out=stats[:, sg, :], in_=x_ap[:, sg * 256:(sg + 1) * 256])
                nc.vector.bn_aggr(out=mvt[:, tt, :], in_=stats[:])

            def ln_rsqrt_batch(mvt, rsv):
                nc.scalar.activation(out=rsv[:], in_=mvt[:, :, 1], func=AF.Ln,
                                     bias=eps_t[:])
                nc.scalar.activation(out=rsv[:], in_=rsv[:], func=AF.Exp, scale=-0.5)

            def ln_apply(x_ap, mvt, rsv, tt, gb):
                nc.vector.tensor_scalar(out=x_ap, in0=x_ap, scalar1=mvt[:, tt, 0:1],
                                        scalar2=rsv[:, tt:tt + 1], op0=ALU.subtract,
                                        op1=ALU.mult)
                if gb is not None:
                    nc.vector.tensor_tensor(out=x_ap, in0=x_ap, in1=gb[0][:], op=ALU.mult)
                    nc.vector.tensor_tensor(out=x_ap, in0=x_ap, in1=gb[1][:], op=ALU.add)

            def ln_all(gb):
                mvt = smp.tile([P, TT, 2], F32, tag="mvt", bufs=2)
                rsv = smp.tile([P, TT], F32, tag="rsv", bufs=2)
                for tt in range(TT):
                    ln_stats(h_res[:, tt, :], mvt, tt)
                ln_rsqrt_batch(mvt, rsv)
                for tt in range(TT):
                    ln_apply(h_res[:, tt, :], mvt, rsv, tt, gb)

            def transpose_to(dst, src_fn):
                """dst [P, FB, T] bf16; src_fn(tt, fb) -> [P, 128] bf16 AP (token-major)."""
                for fb in range(FB):
                    for tg in range(2):
                        pt = ps_mm.tile([P, 512], BF16, tag="mm")
                        for j in range(4):
                            nc.tensor.transpose(out=pt[:, j * P:(j + 1) * P],
                                                in_=src_fn(tg * 4 + j, fb), identity=idbf[:])
                        o = dst[:, fb, tg * 512:(tg + 1) * 512]
                        if (fb * 2 + tg) % 3 == 2:
                            nc.scalar.activation(out=o, in_=pt[:], func=AF.Copy)
                        else:
                            nc.vector.tensor_copy(out=o, in_=pt[:])

            def bias_col(dram, lay, fb):
                c = smp.tile([P, 1], F32, tag="bc")
                nc.sync.dma_start(out=c[:], in_=bass.AP(
                    tensor=dram.tensor, offset=lay * dram.shape[1] * dram.shape[2] + fb * P,
                    ap=[[1, P], [1, 1]]))
                return c

            def bias_field(dram, lay, n):
                f = bigp.tile([P, n], F32, tag="big")
                nc.sync.dma_start(out=f[:], in_=bass.AP(
                    tensor=dram.tensor, offset=lay * n, ap=[[0, P], [1, n]]))
                return f

            # ---- embeddings ----
            for tt in range(TT):
                idx = smp.tile([P, 1], I32, tag="idx")
                nc.sync.dma_start(out=idx[:], in_=tok_d[tt * P:(tt + 1) * P, :])
                tyx = smp.tile([P, 1], I32, tag="tyx")
                nc.sync.dma_start(out=tyx[:], in_=typ_d[tt * P:(tt + 1) * P, :])
                wg = bigp.tile([P, H], BF16, tag="big")
                nc.gpsimd.indirect_dma_start(
                    out=wg[:], out_offset=None, in_=wemb_d[:],
                    in_offset=bass.IndirectOffsetOnAxis(ap=idx[:, :1], axis=0))
                tg_ = bigp.tile([P, H], BF16, tag="big")
                nc.gpsimd.indirect_dma_start(
                    out=tg_[:], out_offset=None, in_=temb_d[:],
                    in_offset=bass.IndirectOffsetOnAxis(ap=tyx[:, :1], axis=0))
                pg = bigp.tile([P, H], BF16, tag="big")
                nc.sync.dma_start(out=pg[:], in_=pemb_d[(tt % 2) * P:(tt % 2 + 1) * P, :])
                t1 = bigp.tile([P, H], F32, tag="big")
                nc.vector.tensor_tensor(out=t1[:], in0=wg[:], in1=pg[:], op=ALU.add)
                t2 = bigp.tile([P, H], F32, tag="big")
                nc.vector.tensor_copy(out=t2[:], in_=tg_[:])
                nc.vector.tensor_tensor(out=h_res[:, tt, :], in0=t1[:], in1=t2[:], op=ALU.add)
            egb = ln_gb_fields(0)
            ln_all(egb)
            if "emb" in tap_d:
                nc.sync.dma_start(out=tap_d["emb"].rearrange("(a p) h -> p a h", p=P), in_=h_res[:])

            # ---- encoder layers ----
            for lay in range(nl):
                hT = bigp.tile([P, FB, T], FP8, tag="big")
                transpose_to(hT, lambda tt, fb: h_res[:, tt, fb * P:(fb + 1) * P])

                def load_proj8(dram, width):
                    ts_ = []
                    for kb2 in range(KB2):
                        w = wpp.tile([P, 2, width], FP8, tag="wp")
                        nc.sync.dma_start(out=w[:], in_=dram[lay, kb2].rearrange(
                            "p (c m) -> p c m", c=2))
                        ts_.append(w)
                    return ts_

                DR = mybir.MatmulPerfMode.DoubleRow
                # q/k feature-major; fp8 DoubleRow, weight stationary across
                # both token halves (one LDWEIGHTS per (fb, kb2)).
                qT = bigp.tile([P, FB, T], BF16, tag="big")
                kT = bigp.tile([P, FB, T], BF16, tag="big")
                for dst, wd, bd in ((qT, wq_d, "bq"), (kT, wk_d, "bk")):
                    wt = load_proj8(wd, H)
                    for fb in range(FB):
                        bc = bias_col({"bq": bq_d, "bk": bk_d}[bd], lay, fb) if use_bias else None
                        pts = [ps_mm.tile([P, 512], F32, tag="mm", name="pts%d" % _i) for _i in range(2)]
                        for kb2 in range(KB2):
                            lw = wt[kb2][:, :, fb * P:(fb + 1) * P]
                            for th in range(2):
                                nc.tensor.matmul(
                                    out=pts[th][:], lhsT=lw,
                                    rhs=hT[:, 2 * kb2:2 * kb2 + 2, th * 512:(th + 1) * 512],
                                    start=(kb2 == 0), stop=(kb2 == KB2 - 1), perf_mode=DR)
                        for th in range(2):
                            o = dst[:, fb, th * 512:(th + 1) * 512]
                            if bc is not None:
                                nc.vector.tensor_scalar(out=o, in0=pts[th][:], scalar1=bc[:],
                                                        scalar2=None, op0=ALU.add)
                            elif th == 0:
                                nc.vector.tensor_copy(out=o, in_=pts[th][:])
                            else:
                                nc.scalar.activation(out=o, in_=pts[th][:], func=AF.Copy)
                # v token-major with ones column; fp8 + padded last dim so
                # the ctx DoubleRow pair step (tt) is 16B-aligned
                DHP = DH + 4
                v4 = bigp.tile([P, TT, NH, DHP], FP8, tag="big")
                nc.vector.memset(v4[:, :, :, DH:DH + 1], 1.0)
                wt = load_proj8(wv_d, H)
                bvf = bias_field(bv_d, lay, H) if use_bias else None
                for tt in range(TT):
                    pvs = [ps_mm.tile([P, 384], F32, tag="mm", name="pvs%d" % _i) for _i in range(2)]
                    for kb2 in range(KB2):
                        lw = hT[:, 2 * kb2:2 * kb2 + 2, tt * P:(tt + 1) * P]
                        for n2 in range(2):
                            nc.tensor.matmul(out=pvs[n2][:], lhsT=lw,
                                             rhs=wt[kb2][:, :, n2 * 384:(n2 + 1) * 384],
                                             start=(kb2 == 0), stop=(kb2 == KB2 - 1),
                                             perf_mode=DR)
                    for n2 in range(2):
                        src = pvs[n2][:].rearrange("p (h d) -> p h d", h=6)
                        if bvf is not None:
                            nc.vector.tensor_tensor(
                                out=v4[:, tt, n2 * 6:(n2 + 1) * 6, 0:DH], in0=src,
                                in1=bvf[:, n2 * 384:(n2 + 1) * 384].rearrange("p (h d) -> p h d", h=6),
                                op=ALU.add)
                        else:
                            nc.vector.tensor_copy(out=v4[:, tt, n2 * 6:(n2 + 1) * 6, 0:DH],
                                                  in_=src)

                # attention per sequence: scores bf16; exp writes fp8 es
                # (scores*SCALE stays in [-2, 2] for this regime); ctx is one
                # fp8 DoubleRow matmul per (b, head, query-half), contracting
                # all 256 keys via the kt pair dim.
                ctx_sb = bigp.tile([P, TT, H], BF16, tag="big")
                cv = ctx_sb[:].rearrange("p t (h d) -> p t h d", h=NH)
                for b in range(BL):
                    for hh in range(2):
                        cps = {qt: ps_cx.tile([P, 6 * (DH + 1)], F32, tag="cx", name="cps")
                               for qt in range(2)}
                        for hi in range(6):
                            h = hh * 6 + hi
                            fb, po = h // 2, (h % 2) * 64
                            st = ps_st.tile([P, 512], F32, tag="st")
                            for kt in range(2):
                                nc.tensor.matmul(
                                    out=st[:, kt * 256:(kt + 1) * 256],
                                    lhsT=kT[po:po + 64, fb,
                                            b * 256 + kt * P: b * 256 + (kt + 1) * P],
                                    rhs=qT[po:po + 64, fb, b * 256:(b + 1) * 256],
                                    start=True, stop=True)
                            es = esp.tile([P, 2, 256], FP8, tag="es")
                            nc.scalar.activation(out=es[:], in_=st[:], func=AF.Exp, scale=SCALE)
                            for qt in range(2):
                                esap = es[:]
                                lw = bass.AP(tensor=esap.tensor,
                                             offset=esap.offset + qt * P,
                                             ap=[list(esap.ap[0]), [256, 2], [1, P]])
                                nc.tensor.matmul(
                                    out=cps[qt][:, hi * (DH + 1):(hi + 1) * (DH + 1)],
                                    lhsT=lw,
                                    rhs=_xap(v4[:, b * 2, h, 0:DH + 1],
                                             [(NH * DHP, 2), (1, DH + 1)]),
                                    start=True, stop=True, perf_mode=DR)
                        for qt in range(2):
                            tt = b * 2 + qt
                            cp = cps[qt][:].rearrange("p (h e) -> p h e", h=6)
                            rt = smp.tile([P, 6], F32, tag="rt")
                            nc.vector.reciprocal(out=rt[:], in_=cp[:, :, DH])
                            nc.vector.tensor_tensor(
                                out=cv[:, tt, hh * 6:(hh + 1) * 6, :], in0=cp[:, :, 0:DH],
                                in1=rt[:, :, None].broadcast_to([P, 6, DH]), op=ALU.mult)

                ctxT = bigp.tile([P, FB, T], FP8, tag="big")
                transpose_to(ctxT, lambda tt, fb: ctx_sb[:, tt, fb * P:(fb + 1) * P])

                # attn output + residual, then LN1
                wt = load_proj8(wo_d, H)
                bof = bias_field(bo_d, lay, H) if use_bias else None
                gb1 = ln_gb_fields(2 * lay + 1)
                mvt1 = smp.tile([P, TT, 2], F32, tag="mvt", bufs=2)
                rsv1 = smp.tile([P, TT], F32, tag="rsv", bufs=2)
                for tt in range(TT):
                    pos = [ps_mm.tile([P, 384], F32, tag="mm", name="pos%d" % _i) for _i in range(2)]
                    for kb2 in range(KB2):
                        lw = ctxT[:, 2 * kb2:2 * kb2 + 2, tt * P:(tt + 1) * P]
                        for n2 in range(2):
                            nc.tensor.matmul(out=pos[n2][:], lhsT=lw,
                                             rhs=wt[kb2][:, :, n2 * 384:(n2 + 1) * 384],
                                             start=(kb2 == 0), stop=(kb2 == KB2 - 1),
                                             perf_mode=DR)
                    for n2 in range(2):
                        sl = h_res[:, tt, n2 * 384:(n2 + 1) * 384]
                        nc.vector.tensor_tensor(out=sl, in0=pos[n2][:], in1=sl, op=ALU.add)
                        if bof is not None:
                            nc.vector.tensor_tensor(out=sl, in0=sl,
                                                    in1=bof[:, n2 * 384:(n2 + 1) * 384], op=ALU.add)
                    ln_stats(h_res[:, tt, :], mvt1, tt)
                ln_rsqrt_batch(mvt1, rsv1)
                for tt in range(TT):
                    ln_apply(h_res[:, tt, :], mvt1, rsv1, tt, gb1)

                # FFN
                h1T = bigp.tile([P, FB, T], FP8, tag="big")
                transpose_to(h1T, lambda tt, fb: h_res[:, tt, fb * P:(fb + 1) * P])
                w1t = []
                for kb2 in range(KB2):
                    w = w1p.tile([P, 2, FF], FP8, tag="w1")
                    nc.sync.dma_start(out=w[:], in_=w1_d[lay, kb2].rearrange(
                        "p (c m) -> p c m", c=2))
                    w1t.append(w)
                w2t = []
                for j in range(FF2):
                    w = w2p.tile([P, 2, H], FP8, tag="w2")
                    nc.sync.dma_start(out=w[:], in_=w2_d[lay, j].rearrange(
                        "p (c m) -> p c m", c=2))
                    w2t.append(w)
                b1c = (lambda fbk: bias_col(b1_d, lay, fbk)) if use_bias else None
                b2f = bias_field(b2_d, lay, H) if use_bias else None
                gb2 = ln_gb_fields(2 * lay + 2)
                g1a = g1p.tile([P, FFB, T], FP8, tag="g1a")
                for fbk in range(FFB):
                    pts = [ps_mm.tile([P, 512], F32, tag="mm", name="pts%d" % _i) for _i in range(2)]
                    for kb2 in range(KB2):
                        lw = w1t[kb2][:, :, fbk * P:(fbk + 1) * P]
                        for th in range(2):
                            nc.tensor.matmul(
                                out=pts[th][:], lhsT=lw,
                                rhs=h1T[:, 2 * kb2:2 * kb2 + 2, th * 512:(th + 1) * 512],
                                start=(kb2 == 0), stop=(kb2 == KB2 - 1), perf_mode=DR)
                    for th in range(2):
                        if b1c is not None:
                            bc = b1c(fbk)
                            nc.vector.tensor_scalar(out=pts[th][:], in0=pts[th][:],
                                                    scalar1=bc[:], scalar2=None, op0=ALU.add)
                        nc.scalar.activation(out=g1a[:, fbk, th * 512:(th + 1) * 512],
                                             in_=pts[th][:], func=AF.Gelu_apprx_tanh)
                for tt in range(TT):
                    pws = [ps_mm.tile([P, 384], F32, tag="mm", name="pws%d" % _i) for _i in range(2)]
                    for j in range(FF2):
                        lw = g1a[:, 2 * j:2 * j + 2, tt * P:(tt + 1) * P]
                        for n2 in range(2):
                            nc.tensor.matmul(out=pws[n2][:], lhsT=lw,
                                             rhs=w2t[j][:, :, n2 * 384:(n2 + 1) * 384],
                                             start=(j == 0), stop=(j == FF2 - 1),
                                             perf_mode=DR)
                    for n2 in range(2):
                        sl = h_res[:, tt, n2 * 384:(n2 + 1) * 384]
                        nc.vector.tensor_tensor(out=sl, in0=pws[n2][:], in1=sl, op=ALU.add)
                        if b2f is not None:
                            nc.vector.tensor_tensor(out=sl, in0=sl,
                                                    in1=b2f[:, n2 * 384:(n2 + 1) * 384],
                                                    op=ALU.add)
                # LN2 batched after both th groups so its Ln/Exp (natlog set)
                # doesn't interleave with Gelu ops (gelu set) on ACT.
                ln_all(gb2)
                if ("layer%d" % lay) in tap_d:
                    nc.sync.dma_start(out=tap_d["layer%d" % lay].rearrange("(a p) h -> p a h", p=P),
                                      in_=h_res[:])

            # ---- emissions: logits + log_softmax ----
            hfT = bigp.tile([P, FB, T], BF16, tag="big")
            transpose_to(hfT, lambda tt, fb: h_res[:, tt, fb * P:(fb + 1) * P])
            for tt in range(TT):
                pt = ps_mm.tile([P, L], F32, tag="mm")
                for kb in range(FB):
                    nc.tensor.matmul(out=pt[:], lhsT=hfT[:, kb, tt * P:(tt + 1) * P],
                                     rhs=fcw_sb[:, kb, :], start=(kb == 0), stop=(kb == FB - 1))
                mx = smp.tile([P, 1], F32, tag="mx")
                nc.vector.tensor_reduce(out=mx[:], in_=pt[:], axis=AX.X, op=ALU.max)
                nmx = smp.tile([P, 1], F32, tag="nmx")
                nc.vector.tensor_scalar(out=nmx[:], in0=mx[:], scalar1=-1.0, scalar2=None,
                                        op0=ALU.mult)
                et = smp.tile([P, L], F32, tag="et")
                nc.scalar.activation(out=et[:], in_=pt[:], func=AF.Exp, bias=nmx[:])
                sm = smp.tile([P, 1], F32, tag="sm")
                nc.vector.tensor_reduce(out=sm[:], in_=et[:], axis=AX.X, op=ALU.add)
                nc.scalar.activation(out=sm[:], in_=sm[:], func=AF.Ln)
                nc.vector.tensor_scalar(out=em_sb[:, tt, :], in0=pt[:], scalar1=mx[:],
                                        scalar2=sm[:], op0=ALU.subtract, op1=ALU.subtract)
            em_d = drp.tile([T, L], F32, tag="em")
            nc.sync.dma_start(out=em_d[:].rearrange("(a p) l -> p a l", p=P), in_=em_sb[:])
            if "em" in tap_d:
                nc.sync.dma_start(out=tap_d["em"].rearrange("(a p) l -> p a l", p=P), in_=em_sb[:])

            # ---- CRF numerator parts ----
            for tt in range(TT):
                yc = smp.tile([P, 1], F32, tag="yc")
                nc.sync.dma_start(out=yc[:], in_=y_d[tt * P:(tt + 1) * P, :])
                yp = smp.tile([P, 1], F32, tag="yp")
                if tt == 0:
                    nc.vector.memset(yp[0:1, :], 0.0)
                    nc.sync.dma_start(out=yp[1:P, :], in_=y_d[0:P - 1, :])
                else:
                    nc.sync.dma_start(out=yp[:], in_=y_d[tt * P - 1:(tt + 1) * P - 1, :])
                tm = smp.tile([P, 1], F32, tag="tm")
                nc.sync.dma_start(out=tm[:], in_=tmask_d[tt * P:(tt + 1) * P, :])
                oh9 = smp.tile([P, L], F32, tag="oh9")
                nc.vector.tensor_scalar(out=oh9[:], in0=iota9[:], scalar1=yc[:], scalar2=None,
                                        op0=ALU.is_equal)
                emy = smp.tile([P, L], F32, tag="emy")
                nc.vector.tensor_tensor(out=emy[:], in0=em_sb[:, tt, :], in1=oh9[:], op=ALU.mult)
                acc = smp.tile([P, 1], F32, tag="acc")
                nc.vector.tensor_reduce(out=acc[:], in_=emy[:], axis=AX.X, op=ALU.add)
                z = smp.tile([P, 1], F32, tag="z")
                nc.scalar.activation(out=z[:], in_=yp[:], func=AF.Copy, scale=float(L))
                nc.vector.tensor_tensor(out=z[:], in0=z[:], in1=yc[:], op=ALU.add)
                oh81 = cmp_.tile([P, L * L], F32, tag="oh81", bufs=3)
                nc.vector.tensor_scalar(out=oh81[:], in0=iota81[:], scalar1=z[:], scalar2=None,
                                        op0=ALU.is_equal)
                nc.vector.tensor_tensor(out=oh81[:], in0=oh81[:], in1=transf[:], op=ALU.mult)
                trs = smp.tile([P, 1], F32, tag="trs")
                nc.vector.tensor_reduce(out=trs[:], in_=oh81[:], axis=AX.X, op=ALU.add)
                nc.vector.tensor_tensor(out=trs[:], in0=trs[:], in1=tm[:], op=ALU.mult)
                nc.vector.tensor_tensor(out=acc[:], in0=acc[:], in1=trs[:], op=ALU.add)
                sef = startf if tt % 2 == 0 else endf
                se = smp.tile([P, L], F32, tag="se")
                nc.vector.tensor_tensor(out=se[:], in0=oh9[:], in1=sef[:], op=ALU.mult)
                se1 = smp.tile([P, 1], F32, tag="se1")
                nc.vector.tensor_reduce(out=se1[:], in_=se[:], axis=AX.X, op=ALU.add)
                nc.vector.tensor_tensor(out=part_all[:, tt:tt + 1], in0=acc[:], in1=se1[:],
                                        op=ALU.add)
            npt = ps_st.tile([1, TT], F32, tag="st")
            nc.tensor.matmul(out=npt[:], lhsT=ones_col[:], rhs=part_all[:], start=True, stop=True)
            nsb = smp.tile([1, TT], F32, tag="nsb")
            nc.scalar.activation(out=nsb[:], in_=npt[:], func=AF.Copy)
            nc.sync.dma_start(out=nums_d[:], in_=nsb[:])

            # ---- CRF denominator: log-semiring reduction tree ----
            # A-side matrices stored flat (i,j); B-side stored TRANSPOSED flat (j,i)
            # so every tensor-op operand AP has <=3 free dims (ISA limit).
            # The tree keeps products packed across partitions so DVE op free
            # sizes shrink with the round (pairs move via SBUF->SBUF DMA).
            L2 = L * L

            def sb_ap(tile_ap, offset, dims):
                return bass.AP(tensor=tile_ap.tensor, offset=tile_ap.offset + offset,
                               ap=[[s, n] for (s, n) in dims])

            def lse_mm_k(A, Bt, Cout, CoutT, pr, k, stable=True):
                """C = A (log-semiring matmul) B, k packed matrices/partition.
                stable=False skips the max-subtraction: legal while entries
                stay well inside exp's f32 range (spans <= 16 steps here)."""
                tmp = bigp.tile([P, BL * L * L * L], F32, tag="big")
                a_ap = _xap(A[0:pr], [(L, k * L), (0, L), (1, L)])
                b_ap = _xap(Bt[0:pr], [(L2, k), (0, L), (1, L2)])
                t3 = _xap(tmp[0:pr], [(L2, k * L), (L, L), (1, L)])
                nc.vector.tensor_tensor(out=t3, in0=a_ap, in1=b_ap, op=ALU.add)
                if stable:
                    mx = cmp_.tile([P, BL, L2], F32, tag="cm")
                    nc.vector.tensor_reduce(out=_xap(mx[0:pr], [(1, k * L2)]),
                                            in_=_xap(tmp[0:pr], [(L, k * L2), (1, L)]),
                                            axis=AX.X, op=ALU.max)
                    mx_b = _xap(mx[0:pr], [(L, k * L), (1, L), (0, L)])
                    nc.vector.tensor_tensor(out=t3, in0=t3, in1=mx_b, op=ALU.subtract)
                nc.scalar.activation(out=_xap(tmp[0:pr], [(1, k * L2 * L)]),
                                     in_=_xap(tmp[0:pr], [(1, k * L2 * L)]), func=AF.Exp)
                sm_ = cmp_.tile([P, BL, L2], F32, tag="cm")
                nc.vector.tensor_reduce(out=_xap(sm_[0:pr], [(1, k * L2)]),
                                        in_=_xap(tmp[0:pr], [(L, k * L2), (1, L)]),
                                        axis=AX.X, op=ALU.add)
                if stable:
                    nc.scalar.activation(out=_xap(sm_[0:pr], [(1, k * L2)]),
                                         in_=_xap(sm_[0:pr], [(1, k * L2)]), func=AF.Ln)
                    nc.vector.tensor_tensor(out=_xap(Cout[0:pr], [(1, k * L2)]),
                                            in0=_xap(sm_[0:pr], [(1, k * L2)]),
                                            in1=_xap(mx[0:pr], [(1, k * L2)]), op=ALU.add)
                    if CoutT is not None:
                        ct_ap = _xap(CoutT[0:pr], [(L2, k), (1, L), (L, L)])
                        s_ap = _xap(sm_[0:pr], [(L2, k), (L, L), (1, L)])
                        m_ap = _xap(mx[0:pr], [(L2, k), (L, L), (1, L)])
                        nc.vector.tensor_tensor(out=ct_ap, in0=s_ap, in1=m_ap,
                                                op=ALU.add)
                else:
                    nc.scalar.activation(out=_xap(Cout[0:pr], [(1, k * L2)]),
                                         in_=_xap(sm_[0:pr], [(1, k * L2)]), func=AF.Ln)
                    if CoutT is not None:
                        nc.scalar.activation(
                            out=_xap(CoutT[0:pr], [(L2, k), (1, L), (L, L)]),
                            in_=_xap(sm_[0:pr], [(L2, k), (L, L), (1, L)]), func=AF.Ln)

            Me = cmp_.tile([P, BL, L2], F32, tag="cm")
            MoT = cmp_.tile([P, BL, L2], F32, tag="cm")
            evod = []
            for par in range(2):
                ev = cmp_.tile([P, BL, L], F32, tag="ev", bufs=4, name="ev")
                for b in range(BL):
                    nc.sync.dma_start(out=ev[:, b, :], in_=bass.AP(
                        tensor=em_d[:].tensor, offset=(b * 256 + par) * L,
                        ap=[[2 * L, P], [1, L]]))
                evod.append(ev)
            nc.vector.tensor_tensor(
                out=Me[:], in0=_xap(transf[:, :], [(0, BL), (1, L2)]),
                in1=_xap(evod[0][:, :, :], [(L, BL), (0, L), (1, L)]), op=ALU.add)
            nc.vector.tensor_tensor(
                out=MoT[:], in0=_xap(transfT[:, :], [(0, BL), (1, L2)]),
                in1=_xap(evod[1][:, :, :], [(L, BL), (1, L), (0, L)]), op=ALU.add)
            nc.sync.dma_start(out=Me[0:1, :, :], in_=bass.AP(
                tensor=pad81_d.tensor, offset=0, ap=[[0, 1], [0, BL], [1, L2]]))

            # R1: 128 products/seq, layout [p=pair, b, 81], k=BL
            Cc = cmp_.tile([P, BL, L2], F32, tag="cm")
            CcT = cmp_.tile([P, BL, L2], F32, tag="cm")
            lse_mm_k(Me, MoT, Cc, CcT, P, BL, stable=False)

            # Shuffles between rounds go through DRAM scratch (partition-
            # crossing strided reads are only supported from DRAM).
            def dram_spill(src, pr, width):
                d = drp.tile([P, BL * L2], F32, tag="cdr", bufs=2)
                nc.sync.dma_start(out=bass.AP(tensor=d[:].tensor, offset=d[:].offset,
                                              ap=[[BL * L2, pr], [1, width]]),
                                  in_=_xap(src[0:pr], [(1, width)]))
                return d

            def dram_row_ap(d, offset, dims):
                return bass.AP(tensor=d[:].tensor, offset=d[:].offset + offset,
                               ap=[[s, n] for (s, n) in dims])

            RW = BL * L2      # dram scratch row width

            # R2: 256 pairs packed [q=(b%2)*64+u, c=b//2]; k=2
            p2 = 2 * L2       # pitch of [P, 2, 81] tiles
            cdr = dram_spill(Cc, P, BL * L2)
            cdrT = dram_spill(CcT, P, BL * L2)
            A2 = cmp_.tile([P, 2, L2], F32, tag="cm")
            Bt2 = cmp_.tile([P, 2, L2], F32, tag="cm")
            for b in range(BL):
                q0 = (b % 2) * 64
                c = b // 2
                nc.sync.dma_start(
                    out=sb_ap(A2[:], q0 * p2 + c * L2, [(p2, 64), (1, L2)]),
                    in_=dram_row_ap(cdr, b * L2, [(2 * RW, 64), (1, L2)]))
                nc.sync.dma_start(
                    out=sb_ap(Bt2[:], q0 * p2 + c * L2, [(p2, 64), (1, L2)]),
                    in_=dram_row_ap(cdrT, RW + b * L2, [(2 * RW, 64), (1, L2)]))
            C2 = cmp_.tile([P, 2, L2], F32, tag="cm")
            C2T = cmp_.tile([P, 2, L2], F32, tag="cm")
            lse_mm_k(A2, Bt2, C2, C2T, P, 2, stable=False)

            # R3: 128 pairs -> [r=b*32+w, 81]; k=1
            cdr = dram_spill(C2, P, 2 * L2)
            cdrT = dram_spill(C2T, P, 2 * L2)
            A3 = cmp_.tile([P, L2], F32, tag="cm")
            Bt3 = cmp_.tile([P, L2], F32, tag="cm")
            for b in range(BL):
                srco = ((b % 2) * 64) * RW + (b // 2) * L2
                nc.sync.dma_start(
                    out=sb_ap(A3[:], (b * 32) * L2, [(L2, 32), (1, L2)]),
                    in_=dram_row_ap(cdr, srco, [(2 * RW, 32), (1, L2)]))
                nc.sync.dma_start(
                    out=sb_ap(Bt3[:], (b * 32) * L2, [(L2, 32), (1, L2)]),
                    in_=dram_row_ap(cdrT, srco + RW, [(2 * RW, 32), (1, L2)]))
            Cr = cmp_.tile([P, L2], F32, tag="cm")
            CrT = cmp_.tile([P, L2], F32, tag="cm")
            lse_mm_k(A3, Bt3, Cr, CrT, P, 1, stable=False)

            # R4..R6: products (b, x) at partition b*npp+x; pair via one
            # strided read per side. Stop at 4 products/seq (span 64).
            for npp, stb in ((32, False), (16, True), (8, True)):
                pr = BL * npp // 2
                cdr = dram_spill(Cr, 2 * pr, L2)
                cdrT = dram_spill(CrT, 2 * pr, L2)
                Ax = cmp_.tile([P, L2], F32, tag="cm")
                Btx = cmp_.tile([P, L2], F32, tag="cm")
                nc.sync.dma_start(out=sb_ap(Ax[:], 0, [(L2, pr), (1, L2)]),
                                  in_=dram_row_ap(cdr, 0, [(2 * RW, pr), (1, L2)]))
                nc.sync.dma_start(out=sb_ap(Btx[:], 0, [(L2, pr), (1, L2)]),
                                  in_=dram_row_ap(cdrT, RW, [(2 * RW, pr), (1, L2)]))
                Cr = cmp_.tile([P, L2], F32, tag="cm")
                CrT = cmp_.tile([P, L2], F32, tag="cm")
                lse_mm_k(Ax, Btx, Cr, CrT, pr, 1, stable=stb)

            # alpha chain over the 4 remaining span-64 factors per sequence,
            # all on partitions 0..3 (one per sequence).
            em0b = cmp_.tile([P, L], F32, tag="ev", bufs=4)
            nc.sync.dma_start(out=em0b[0:BL, :], in_=bass.AP(
                tensor=em_d[:].tensor, offset=0, ap=[[S * L, BL], [1, L]]))
            st4 = cmp_.tile([P, L], F32, tag="ev", bufs=4)
            nc.sync.dma_start(out=st4[0:BL, :], in_=bass.AP(
                tensor=start_d.tensor, offset=0, ap=[[0, BL], [1, L]]))
            en4 = cmp_.tile([P, L], F32, tag="ev", bufs=4)
            nc.sync.dma_start(out=en4[0:BL, :], in_=bass.AP(
                tensor=end_d.tensor, offset=0, ap=[[0, BL], [1, L]]))
            al = cmp_.tile([P, L], F32, tag="ev", bufs=4)
            nc.vector.tensor_tensor(out=al[0:BL, :], in0=em0b[0:BL, :],
                                    in1=st4[0:BL, :], op=ALU.add)
            cdrT = dram_spill(CrT, 4 * BL, L2)
            for z in range(4):
                Pz = cmp_.tile([P, L2], F32, tag="cm")
                nc.sync.dma_start(out=sb_ap(Pz[:], 0, [(L2, BL), (1, L2)]),
                                  in_=dram_row_ap(cdrT, z * RW, [(4 * RW, BL), (1, L2)]))
                t3z = cmp_.tile([P, L2], F32, tag="cm")
                nc.vector.tensor_tensor(out=_xap(t3z[0:BL], [(L, L), (1, L)]),
                                        in0=_xap(al[0:BL], [(0, L), (1, L)]),
                                        in1=_xap(Pz[0:BL], [(L, L), (1, L)]),
                                        op=ALU.add)
                mxz = cmp_.tile([P, L], F32, tag="ev", bufs=4)
                nc.vector.tensor_reduce(out=_xap(mxz[0:BL], [(1, L)]),
                                        in_=_xap(t3z[0:BL], [(L, L), (1, L)]),
                                        axis=AX.X, op=ALU.max)
                nc.vector.tensor_tensor(out=_xap(t3z[0:BL], [(L, L), (1, L)]),
                                        in0=_xap(t3z[0:BL], [(L, L), (1, L)]),
                                        in1=_xap(mxz[0:BL], [(1, L), (0, L)]),
                                        op=ALU.subtract)
                nc.scalar.activation(out=t3z[0:BL, :], in_=t3z[0:BL, :], func=AF.Exp)
                al = cmp_.tile([P, L], F32, tag="ev", bufs=4)
                nc.vector.tensor_reduce(out=_xap(al[0:BL], [(1, L)]),
                                        in_=_xap(t3z[0:BL], [(L, L), (1, L)]),
                                        axis=AX.X, op=ALU.add)
                nc.scalar.activation(out=al[0:BL, :], in_=al[0:BL, :], func=AF.Ln)
                nc.vector.tensor_tensor(out=al[0:BL, :], in0=al[0:BL, :],
                                        in1=mxz[0:BL, :], op=ALU.add)

            # den[b] = lse_j(al[b, j] + end[j])
            nc.vector.tensor_tensor(out=al[0:BL, :], in0=al[0:BL, :],
                                    in1=en4[0:BL, :], op=ALU.add)
            dmx = smp.tile([P, 1], F32, tag="dmx")
            nc.vector.tensor_reduce(out=dmx[0:BL, :], in_=al[0:BL, :], axis=AX.X,
                                    op=ALU.max)
            nc.vector.tensor_scalar(out=al[0:BL, :], in0=al[0:BL, :],
                                    scalar1=dmx[0:BL, :], scalar2=None, op0=ALU.subtract)
            nc.scalar.activation(out=al[0:BL, :], in_=al[0:BL, :], func=AF.Exp)
            dsm = smp.tile([P, 1], F32, tag="dsm")
            nc.vector.tensor_reduce(out=dsm[0:BL, :], in_=al[0:BL, :], axis=AX.X,
                                    op=ALU.add)
            nc.scalar.activation(out=dsm[0:BL, :], in_=dsm[0:BL, :], func=AF.Ln)
            nc.vector.tensor_tensor(out=dsm[0:BL, :], in0=dsm[0:BL, :],
                                    in1=dmx[0:BL, :], op=ALU.add)
            nc.sync.dma_start(
                out=bass.AP(tensor=dens_d.tensor, offset=0, ap=[[1, BL], [1, 1]]),
                in_=dsm[0:BL, :])

    nc.compile()
    return nc


_PROG = {}


def get_program(nl=NL, taps=(), use_bias=False, use_lngb=False):
    key = (nl, tuple(taps), use_bias, use_lngb)
    if key not in _PROG:
        _PROG[key] = build(nl, taps, use_bias, use_lngb)
    return _PROG[key]


def prepare_in_maps(inputs, nl=NL, use_bias=False, use_lngb=False):
    """Shard + preprocess host inputs into per-core in_maps."""
    g = {k: np.asarray(v) for k, v in inputs.items()}
    assert np.all(g["attention_mask"] == 1), "kernel specialized for all-ones attention_mask"
    bf = lambda a: np.ascontiguousarray(a.astype(BF16NP))
    f32 = lambda a: np.ascontiguousarray(a.astype(np.float32))

    def pairw(W):
        """[nl, K, M] -> fp8 [nl, K//256, 128, 2*M]: row r = kb2*256 + c*128 + p."""
        Wn = np.clip(np.asarray(W, np.float32), -240.0, 240.0)
        nlx, Kd, Md = Wn.shape
        Wn = Wn.reshape(nlx, Kd // 256, 2, 128, Md).transpose(0, 1, 3, 2, 4)
        return np.ascontiguousarray(
            Wn.reshape(nlx, Kd // 256, 128, 2 * Md).astype(ml_dtypes.float8_e4m3fn))

    shared = {
        "wemb": bf(g["word_emb"]), "pemb": bf(g["pos_emb"]), "temb": bf(g["type_emb"]),
        "wq": pairw(g["Wq"][:nl]), "wk": pairw(g["Wk"][:nl]), "wv": pairw(g["Wv"][:nl]),
        "wo": pairw(g["Wo"][:nl]), "w1": pairw(g["W1"][:nl]), "w2": pairw(g["W2"][:nl]),
        "fcw": bf(g["fc_w"]),
        "ctrans": f32(g["crf_trans"].reshape(1, L * L)),
        "ctransT": f32(np.ascontiguousarray(np.asarray(g["crf_trans"]).T).reshape(1, L * L)),
        "cstart": f32(g["crf_start"].reshape(1, L)),
        "cend": f32(g["crf_end"].reshape(1, L)),
        "iota9": np.broadcast_to(np.arange(L, dtype=np.float32), (P, L)).copy(),
        "iota81": np.broadcast_to(np.arange(L * L, dtype=np.float32), (P, L * L)).copy(),
        "tmask": np.ascontiguousarray(
            (np.arange(T) % S != 0).astype(np.float32).reshape(T, 1)),
        "pad81": np.where(np.eye(L, dtype=np.float32).reshape(1, L * L) > 0, 0.0,
                          np.float32(NEG)).astype(np.float32),
    }
    if use_bias:
        shared.update({
            "bq": f32(g["bq"][:nl, None, :]), "bk": f32(g["bk"][:nl, None, :]),
            "bv": f32(g["bv"][:nl, None, :]), "bo": f32(g["bo"][:nl, None, :]),
            "b1": f32(g["b1"][:nl, None, :]), "b2": f32(g["b2"][:nl, None, :])})
    if use_lngb:
        lng = np.concatenate([g["emb_ln_g"][None],
                              np.stack([g["ln1_g"], g["ln2_g"]], 1).reshape(2 * nl, H)])[:, None, :]
        lnb = np.concatenate([g["emb_ln_b"][None],
                              np.stack([g["ln1_b"], g["ln2_b"]], 1).reshape(2 * nl, H)])[:, None, :]
        shared.update({"lng": f32(lng), "lnb": f32(lnb)})
    in_maps = []
    for c in range(NCORES):
        sl = slice(c * BL, (c + 1) * BL)
        m = dict(shared)
        m["tok"] = np.ascontiguousarray(g["token_ids"][sl].reshape(T, 1).astype(np.int32))
        m["typ"] = np.ascontiguousarray(g["token_type_ids"][sl].reshape(T, 1).astype(np.int32))
        m["y"] = np.ascontiguousarray(g["y"][sl].reshape(T, 1).astype(np.float32))
        in_maps.append(m)
    return in_maps


def needs_general(inputs):
    g = inputs
    use_bias = any(np.asarray(g[k]).any() for k in ("bq", "bk", "bv", "bo", "b1", "b2"))
    use_lngb = (not np.all(np.asarray(g["ln1_g"]) == 1) or np.asarray(g["ln1_b"]).any()
                or not np.all(np.asarray(g["ln2_g"]) == 1) or np.asarray(g["ln2_b"]).any()
                or not np.all(np.asarray(g["emb_ln_g"]) == 1) or np.asarray(g["emb_ln_b"]).any())
    return use_bias, use_lngb


def loss_from_results(results):
    tot = 0.0
    for r in results:
        tot += float(np.asarray(r["dens"], np.float64).sum()
                     - np.asarray(r["nums"], np.float64).sum())
    return np.float32(tot)


def kernel(**inputs):
    use_bias, use_lngb = needs_general(inputs)
    nc = get_program(NL, (), use_bias, use_lngb)
    in_maps = prepare_in_maps(inputs, NL, use_bias, use_lngb)
    res = run_bass_kernel_spmd(nc, in_maps, list(range(NCORES)))
    return loss_from_results(res.results)



# revision 32
# speedup vs baseline: 1.0364x; 1.0364x over previous
"""Trainium2 Bass kernel: BERT(12L)+CRF loss, data-parallel over batch on 8 cores.

Self-contained: hardcodes shapes from the problem spec. Each core processes 4
sequences end-to-end on device (embeddings, 12 encoder layers, log-softmax
emissions, CRF numerator + log-partition via a log-semiring reduction tree) and
emits per-sequence partial sums; the host sums 8 cores' partials into the
scalar loss.
"""

import sys

sys.path.insert(0, "/opt/trn_rl_repo")

import numpy as np
import ml_dtypes

import concourse.bass as bass
import concourse.bacc as bacc
import concourse.mybir as mybir
import concourse.tile as tile
from concourse.bass_utils import run_bass_kernel_spmd
from concourse.masks import make_identity

B, S, V, H, NL, NH, FF, L = 32, 256, 30522, 768, 12, 12, 3072, 9
DH = H // NH
SCALE = 1.0 / float(np.sqrt(DH))
NCORES = 8
BL = B // NCORES          # 4 sequences per core
T = BL * S                # 1024 tokens per core
P = 128
TT = T // P               # 8 token tiles
FB = H // P               # 6 feature blocks
FFB = FF // P             # 24 ff blocks
NEG = -30.0               # log-semiring pad "-inf"
LNEPS = 1e-12

F32 = mybir.dt.float32
BF16 = mybir.dt.bfloat16
FP8 = mybir.dt.float8e4
I32 = mybir.dt.int32
AF = mybir.ActivationFunctionType
ALU = mybir.AluOpType
AX = mybir.AxisListType
BF16NP = ml_dtypes.bfloat16


def _xap(ap, dims):
    """Raw AP with the partition dim of `ap` and explicit free dims [(step, count)...]."""
    return bass.AP(tensor=ap.tensor, offset=ap.offset,
                   ap=[list(ap.ap[0])] + [[s, n] for (s, n) in dims])


def _steer_act_tables(nc):
    """Steer ACT table-set selection: route Exp and Ln to the combined
    natural_log_exp set so the CRF tail doesn't ping-pong between the
    exp-only and ln-only sets (each switch costs ~2.7us on hardware)."""
    from concourse.hw_specs import get_activation_tables
    try:
        tabs = get_activation_tables(nc.m.arch)
    except Exception:
        return
    both = tabs.get("natural_log_exp_and_others")
    if not both or AF.Exp not in both:
        return
    ex = tabs.get("exp_and_others")
    if ex is not None:
        ex.discard(AF.Exp)
    nl_ = tabs.get("natural_log")
    if nl_ is not None:
        nl_.discard(AF.Ln)


def build(nl=NL, taps=(), use_bias=False, use_lngb=False):
    """Emit the full per-core program. Returns (nc, tap_names)."""
    nc = bacc.Bacc("TRN2", target_bir_lowering=False, debug=False)
    _steer_act_tables(nc)
    dt_in = lambda name, shape, dt: nc.dram_tensor(name, shape, dt, kind="ExternalInput").ap()
    dt_out = lambda name, shape, dt: nc.dram_tensor(name, shape, dt, kind="ExternalOutput").ap()

    tok_d = dt_in("tok", [T, 1], I32)
    typ_d = dt_in("typ", [T, 1], I32)
    y_d = dt_in("y", [T, 1], F32)
    wemb_d = dt_in("wemb", [V, H], BF16)
    pemb_d = dt_in("pemb", [S, H], BF16)
    temb_d = dt_in("temb", [2, H], BF16)
    KB2 = H // 256      # contract pair-chunks over H (DoubleRow fp8)
    FF2 = FF // 256     # contract pair-chunks over FF
    wq_d = dt_in("wq", [nl, KB2, P, 2 * H], FP8)
    wk_d = dt_in("wk", [nl, KB2, P, 2 * H], FP8)
    wv_d = dt_in("wv", [nl, KB2, P, 2 * H], FP8)
    wo_d = dt_in("wo", [nl, KB2, P, 2 * H], FP8)
    w1_d = dt_in("w1", [nl, KB2, P, 2 * FF], FP8)
    w2_d = dt_in("w2", [nl, FF2, P, 2 * H], FP8)
    fcw_d = dt_in("fcw", [H, L], BF16)
    trans_d = dt_in("ctrans", [1, L * L], F32)
    transT_d = dt_in("ctransT", [1, L * L], F32)
    start_d = dt_in("cstart", [1, L], F32)
    end_d = dt_in("cend", [1, L], F32)
    iota9_d = dt_in("iota9", [P, L], F32)
    iota81_d = dt_in("iota81", [P, L * L], F32)
    tmask_d = dt_in("tmask", [T, 1], F32)
    pad81_d = dt_in("pad81", [1, L * L], F32)
    if use_bias:
        bq_d = dt_in("bq", [nl, 1, H], F32)
        bk_d = dt_in("bk", [nl, 1, H], F32)
        bv_d = dt_in("bv", [nl, 1, H], F32)
        bo_d = dt_in("bo", [nl, 1, H], F32)
        b1_d = dt_in("b1", [nl, 1, FF], F32)
        b2_d = dt_in("b2", [nl, 1, H], F32)
    if use_lngb:
        lng_d = dt_in("lng", [2 * nl + 1, 1, H], F32)  # emb, (ln1,ln2)*nl
        lnb_d = dt_in("lnb", [2 * nl + 1, 1, H], F32)

    nums_d = dt_out("nums", [1, TT], F32)
    dens_d = dt_out("dens", [1, BL], F32)
    tap_d = {}
    for tp in taps:
        if tp == "em":
            tap_d[tp] = dt_out("tap_em", [T, L], F32)
        else:
            tap_d[tp] = dt_out("tap_" + tp, [T, H], F32)

    with tile.TileContext(nc) as tc:
        from contextlib import ExitStack
        with ExitStack() as ctx:
            pers = ctx.enter_context(tc.tile_pool(name="pers", bufs=1))
            bigp = ctx.enter_context(tc.tile_pool(name="bigp", bufs=4))
            g1p = ctx.enter_context(tc.tile_pool(name="g1p", bufs=1))
            wpp = ctx.enter_context(tc.tile_pool(name="wpp", bufs=14))
            w1p = ctx.enter_context(tc.tile_pool(name="w1p", bufs=4))
            w2p = ctx.enter_context(tc.tile_pool(name="w2p", bufs=14))
            esp = ctx.enter_context(tc.tile_pool(name="esp", bufs=4))
            smp = ctx.enter_context(tc.tile_pool(name="smp", bufs=4))
            cmp_ = ctx.enter_context(tc.tile_pool(name="cmp", bufs=6))
            ps_mm = ctx.enter_context(tc.tile_pool(name="ps_mm", bufs=4, space="PSUM"))
            ps_st = ctx.enter_context(tc.tile_pool(name="ps_st", bufs=2, space="PSUM"))
            ps_cx = ctx.enter_context(tc.tile_pool(name="ps_cx", bufs=2, space="PSUM"))
            drp = ctx.enter_context(tc.tile_pool(name="drp", bufs=1, space="DRAM"))

            # ---- constants ----
            idbf = pers.tile([P, P], BF16)
            make_identity(nc, idbf[:])
            ones_col = pers.tile([P, 1], F32)
            nc.vector.memset(ones_col[:], 1.0)
            eps_t = pers.tile([P, 1], F32)
            nc.vector.memset(eps_t[:], LNEPS)
            iota9 = pers.tile([P, L], F32)
            nc.sync.dma_start(out=iota9[:], in_=iota9_d[:])
            iota81 = pers.tile([P, L * L], F32)
            nc.sync.dma_start(out=iota81[:], in_=iota81_d[:])
            transf = pers.tile([P, L * L], F32)
            nc.sync.dma_start(out=transf[:], in_=bass.AP(
                tensor=trans_d.tensor, offset=0, ap=[[0, P], [1, L * L]]))
            transfT = pers.tile([P, L * L], F32)
            nc.sync.dma_start(out=transfT[:], in_=bass.AP(
                tensor=transT_d.tensor, offset=0, ap=[[0, P], [1, L * L]]))
            startf = pers.tile([P, L], F32)
            nc.vector.memset(startf[:], 0.0)
            nc.sync.dma_start(out=startf[0:1, :], in_=start_d[:])
            endf = pers.tile([P, L], F32)
            nc.vector.memset(endf[:], 0.0)
            nc.sync.dma_start(out=endf[P - 1:P, :], in_=end_d[:])
            fcw_sb = pers.tile([P, FB, L], BF16)
            nc.sync.dma_start(out=fcw_sb[:], in_=fcw_d[:].rearrange("(kb p) l -> p kb l", p=P))

            h_res = pers.tile([P, TT, H], BF16)
            em_sb = pers.tile([P, TT, L], F32)
            part_all = pers.tile([P, TT], F32)

            def ln_gb_fields(idx):
                if not use_lngb:
                    return None
                gf = bigp.tile([P, H], F32, tag="big")
                bf = bigp.tile([P, H], F32, tag="big")
                nc.sync.dma_start(out=gf[:], in_=bass.AP(
                    tensor=lng_d.tensor, offset=idx * H, ap=[[0, P], [1, H]]))
                nc.sync.dma_start(out=bf[:], in_=bass.AP(
                    tensor=lnb_d.tensor, offset=idx * H, ap=[[0, P], [1, H]]))
                return gf, bf

            # LayerNorm split into per-tile stats, one batched rsqrt for all 8
            # tiles (2 tiny ACT ops in the natural_log_exp set -> no table
            # switches near Gelu), then per-tile apply.
            def ln_stats(x_ap, mvt, tt):
                stats = smp.tile([P, 3, 6], F32, tag="st9")
                for sg in range(3):
                    nc.vector.bn_stats(out=stats[:, sg, :], in_=x_ap[:, sg * 256:(sg + 1) * 256])
                nc.vector.bn_aggr(out=mvt[:, tt, :], in_=stats[:])

            def ln_rsqrt_batch(mvt, rsv):
                nc.scalar.activation(out=rsv[:], in_=mvt[:, :, 1], func=AF.Ln,
                                     bias=eps_t[:])
                nc.scalar.activation(out=rsv[:], in_=rsv[:], func=AF.Exp, scale=-0.5)

            def ln_apply(x_ap, mvt, rsv, tt, gb):
                nc.vector.tensor_scalar(out=x_ap, in0=x_ap, scalar1=mvt[:, tt, 0:1],
                                        scalar2=rsv[:, tt:tt + 1], op0=ALU.subtract,
                                        op1=ALU.mult)
                if gb is not None:
                    nc.vector.tensor_tensor(out=x_ap, in0=x_ap, in1=gb[0][:], op=ALU.mult)
                    nc.vector.tensor_tensor(out=x_ap, in0=x_ap, in1=gb[1][:], op=ALU.add)

            def ln_all(gb):
                mvt = smp.tile([P, TT, 2], F32, tag="mvt", bufs=2)
                rsv = smp.tile([P, TT], F32, tag="rsv", bufs=2)
                for tt in range(TT):
                    ln_stats(h_res[:, tt, :], mvt, tt)
                ln_rsqrt_batch(mvt, rsv)
                for tt in range(TT):
                    ln_apply(h_res[:, tt, :], mvt, rsv, tt, gb)

            def transpose_to(dst, src_fn):
                """dst [P, FB, T] bf16; src_fn(tt, fb) -> [P, 128] bf16 AP (token-major)."""
                for fb in range(FB):
                    for tg in range(2):
                        pt = ps_mm.tile([P, 512], BF16, tag="mm")
                        for j in range(4):
                            nc.tensor.transpose(out=pt[:, j * P:(j + 1) * P],
                                                in_=src_fn(tg * 4 + j, fb), identity=idbf[:])
                        o = dst[:, fb, tg * 512:(tg + 1) * 512]
                        if (fb * 2 + tg) % 3 == 0:
                            nc.scalar.activation(out=o, in_=pt[:], func=AF.Copy)
                        else:
                            nc.vector.tensor_copy(out=o, in_=pt[:])

            def bias_col(dram, lay, fb):
                c = smp.tile([P, 1], F32, tag="bc")
                nc.sync.dma_start(out=c[:], in_=bass.AP(
                    tensor=dram.tensor, offset=lay * dram.shape[1] * dram.shape[2] + fb * P,
                    ap=[[1, P], [1, 1]]))
                return c

            def bias_field(dram, lay, n):
                f = bigp.tile([P, n], F32, tag="big")
                nc.sync.dma_start(out=f[:], in_=bass.AP(
                    tensor=dram.tensor, offset=lay * n, ap=[[0, P], [1, n]]))
                return f

            # ---- embeddings ----
            for tt in range(TT):
                idx = smp.tile([P, 1], I32, tag="idx")
                nc.sync.dma_start(out=idx[:], in_=tok_d[tt * P:(tt + 1) * P, :])
                tyx = smp.tile([P, 1], I32, tag="tyx")
                nc.sync.dma_start(out=tyx[:], in_=typ_d[tt * P:(tt + 1) * P, :])
                wg = bigp.tile([P, H], BF16, tag="big")
                nc.gpsimd.indirect_dma_start(
                    out=wg[:], out_offset=None, in_=wemb_d[:],
                    in_offset=bass.IndirectOffsetOnAxis(ap=idx[:, :1], axis=0))
                tg_ = bigp.tile([P, H], BF16, tag="big")
                nc.gpsimd.indirect_dma_start(
                    out=tg_[:], out_offset=None, in_=temb_d[:],
                    in_offset=bass.IndirectOffsetOnAxis(ap=tyx[:, :1], axis=0))
                pg = bigp.tile([P, H], BF16, tag="big")
                nc.sync.dma_start(out=pg[:], in_=pemb_d[(tt % 2) * P:(tt % 2 + 1) * P, :])
                t1 = bigp.tile([P, H], F32, tag="big")
                nc.vector.tensor_tensor(out=t1[:], in0=wg[:], in1=pg[:], op=ALU.add)
                t2 = bigp.tile([P, H], F32, tag="big")
                nc.vector.tensor_copy(out=t2[:], in_=tg_[:])
                nc.vector.tensor_tensor(out=h_res[:, tt, :], in0=t1[:], in1=t2[:], op=ALU.add)
            egb = ln_gb_fields(0)
            ln_all(egb)
            if "emb" in tap_d:
                nc.sync.dma_start(out=tap_d["emb"].rearrange("(a p) h -> p a h", p=P), in_=h_res[:])

            # ---- encoder layers ----
            for lay in range(nl):
                hT = bigp.tile([P, FB, T], FP8, tag="big")
                transpose_to(hT, lambda tt, fb: h_res[:, tt, fb * P:(fb + 1) * P])

                def load_proj8(dram, width):
                    ts_ = []
                    for kb2 in range(KB2):
                        w = wpp.tile([P, 2, width], FP8, tag="wp")
                        nc.sync.dma_start(out=w[:], in_=dram[lay, kb2].rearrange(
                            "p (c m) -> p c m", c=2))
                        ts_.append(w)
                    return ts_

                DR = mybir.MatmulPerfMode.DoubleRow
                # q/k feature-major; fp8 DoubleRow, weight stationary across
                # both token halves (one LDWEIGHTS per (fb, kb2)).
                qT = bigp.tile([P, FB, T], BF16, tag="big")
                kT = bigp.tile([P, FB, T], BF16, tag="big")
                for dst, wd, bd in ((qT, wq_d, "bq"), (kT, wk_d, "bk")):
                    wt = load_proj8(wd, H)
                    for fb in range(FB):
                        bc = bias_col({"bq": bq_d, "bk": bk_d}[bd], lay, fb) if use_bias else None
                        pts = [ps_mm.tile([P, 512], F32, tag="mm", name="pts%d" % _i) for _i in range(2)]
                        for kb2 in range(KB2):
                            lw = wt[kb2][:, :, fb * P:(fb + 1) * P]
                            for th in range(2):
                                nc.tensor.matmul(
                                    out=pts[th][:], lhsT=lw,
                                    rhs=hT[:, 2 * kb2:2 * kb2 + 2, th * 512:(th + 1) * 512],
                                    start=(kb2 == 0), stop=(kb2 == KB2 - 1), perf_mode=DR)
                        for th in range(2):
                            o = dst[:, fb, th * 512:(th + 1) * 512]
                            if bc is not None:
                                nc.vector.tensor_scalar(out=o, in0=pts[th][:], scalar1=bc[:],
                                                        scalar2=None, op0=ALU.add)
                            elif th == 0:
                                nc.vector.tensor_copy(out=o, in_=pts[th][:])
                            else:
                                nc.scalar.activation(out=o, in_=pts[th][:], func=AF.Copy)
                # v token-major with ones column
                v4 = bigp.tile([P, TT, NH, DH + 1], BF16, tag="big")
                nc.vector.memset(v4[:, :, :, DH:DH + 1], 1.0)
                wt = load_proj8(wv_d, H)
                bvf = bias_field(bv_d, lay, H) if use_bias else None
                for tt in range(TT):
                    pvs = [ps_mm.tile([P, 384], F32, tag="mm", name="pvs%d" % _i) for _i in range(2)]
                    for kb2 in range(KB2):
                        lw = hT[:, 2 * kb2:2 * kb2 + 2, tt * P:(tt + 1) * P]
                        for n2 in range(2):
                            nc.tensor.matmul(out=pvs[n2][:], lhsT=lw,
                                             rhs=wt[kb2][:, :, n2 * 384:(n2 + 1) * 384],
                                             start=(kb2 == 0), stop=(kb2 == KB2 - 1),
                                             perf_mode=DR)
                    for n2 in range(2):
                        src = pvs[n2][:].rearrange("p (h d) -> p h d", h=6)
                        if bvf is not None:
                            nc.vector.tensor_tensor(
                                out=v4[:, tt, n2 * 6:(n2 + 1) * 6, 0:DH], in0=src,
                                in1=bvf[:, n2 * 384:(n2 + 1) * 384].rearrange("p (h d) -> p h d", h=6),
                                op=ALU.add)
                        else:
                            nc.vector.tensor_copy(out=v4[:, tt, n2 * 6:(n2 + 1) * 6, 0:DH],
                                                  in_=src)

                # attention per sequence
                ctx_sb = bigp.tile([P, TT, H], BF16, tag="big")
                cv = ctx_sb[:].rearrange("p t (h d) -> p t h d", h=NH)
                for b in range(BL):
                    for hh in range(2):
                        cps = {qt: ps_cx.tile([P, 6 * (DH + 1)], F32, tag="cx", name="cps")
                               for qt in range(2)}
                        for hi in range(6):
                            h = hh * 6 + hi
                            fb, po = h // 2, (h % 2) * 64
                            st = ps_st.tile([P, 512], F32, tag="st")
                            for kt in range(2):
                                nc.tensor.matmul(
                                    out=st[:, kt * 256:(kt + 1) * 256],
                                    lhsT=kT[po:po + 64, fb,
                                            b * 256 + kt * P: b * 256 + (kt + 1) * P],
                                    rhs=qT[po:po + 64, fb, b * 256:(b + 1) * 256],
                                    start=True, stop=True)
                            es = esp.tile([P, 512], BF16, tag="es")
                            nc.scalar.activation(out=es[:], in_=st[:], func=AF.Exp, scale=SCALE)
                            for qt in range(2):
                                for kt in range(2):
                                    nc.tensor.matmul(
                                        out=cps[qt][:, hi * (DH + 1):(hi + 1) * (DH + 1)],
                                        lhsT=es[:, kt * 256 + qt * P: kt * 256 + (qt + 1) * P],
                                        rhs=v4[:, b * 2 + kt, h, :],
                                        start=(kt == 0), stop=(kt == 1))
                        for qt in range(2):
                            tt = b * 2 + qt
                            cp = cps[qt][:].rearrange("p (h e) -> p h e", h=6)
                            rt = smp.tile([P, 6], F32, tag="rt")
                            nc.vector.reciprocal(out=rt[:], in_=cp[:, :, DH])
                            nc.vector.tensor_tensor(
                                out=cv[:, tt, hh * 6:(hh + 1) * 6, :], in0=cp[:, :, 0:DH],
                                in1=rt[:, :, None].broadcast_to([P, 6, DH]), op=ALU.mult)

                ctxT = bigp.tile([P, FB, T], FP8, tag="big")
                transpose_to(ctxT, lambda tt, fb: ctx_sb[:, tt, fb * P:(fb + 1) * P])

                # attn output + residual, then LN1
                wt = load_proj8(wo_d, H)
                bof = bias_field(bo_d, lay, H) if use_bias else None
                gb1 = ln_gb_fields(2 * lay + 1)
                mvt1 = smp.tile([P, TT, 2], F32, tag="mvt", bufs=2)
                rsv1 = smp.tile([P, TT], F32, tag="rsv", bufs=2)
                for tt in range(TT):
                    pos = [ps_mm.tile([P, 384], F32, tag="mm", name="pos%d" % _i) for _i in range(2)]
                    for kb2 in range(KB2):
                        lw = ctxT[:, 2 * kb2:2 * kb2 + 2, tt * P:(tt + 1) * P]
                        for n2 in range(2):
                            nc.tensor.matmul(out=pos[n2][:], lhsT=lw,
                                             rhs=wt[kb2][:, :, n2 * 384:(n2 + 1) * 384],
                                             start=(kb2 == 0), stop=(kb2 == KB2 - 1),
                                             perf_mode=DR)
                    for n2 in range(2):
                        sl = h_res[:, tt, n2 * 384:(n2 + 1) * 384]
                        nc.vector.tensor_tensor(out=sl, in0=pos[n2][:], in1=sl, op=ALU.add)
                        if bof is not None:
                            nc.vector.tensor_tensor(out=sl, in0=sl,
                                                    in1=bof[:, n2 * 384:(n2 + 1) * 384], op=ALU.add)
                    ln_stats(h_res[:, tt, :], mvt1, tt)
                ln_rsqrt_batch(mvt1, rsv1)
                for tt in range(TT):
                    ln_apply(h_res[:, tt, :], mvt1, rsv1, tt, gb1)

                # FFN
                h1T = bigp.tile([P, FB, T], FP8, tag="big")
                transpose_to(h1T, lambda tt, fb: h_res[:, tt, fb * P:(fb + 1) * P])
                w1t = []
                for kb2 in range(KB2):
                    w = w1p.tile([P, 2, FF], FP8, tag="w1")
                    nc.sync.dma_start(out=w[:], in_=w1_d[lay, kb2].rearrange(
                        "p (c m) -> p c m", c=2))
                    w1t.append(w)
                w2t = []
                for j in range(FF2):
                    w = w2p.tile([P, 2, H], FP8, tag="w2")
                    nc.sync.dma_start(out=w[:], in_=w2_d[lay, j].rearrange(
                        "p (c m) -> p c m", c=2))
                    w2t.append(w)
                b1c = (lambda fbk: bias_col(b1_d, lay, fbk)) if use_bias else None
                b2f = bias_field(b2_d, lay, H) if use_bias else None
                gb2 = ln_gb_fields(2 * lay + 2)
                g1a = g1p.tile([P, FFB, T], FP8, tag="g1a")
                for fbk in range(FFB):
                    pts = [ps_mm.tile([P, 512], F32, tag="mm", name="pts%d" % _i) for _i in range(2)]
                    for kb2 in range(KB2):
                        lw = w1t[kb2][:, :, fbk * P:(fbk + 1) * P]
                        for th in range(2):
                            nc.tensor.matmul(
                                out=pts[th][:], lhsT=lw,
                                rhs=h1T[:, 2 * kb2:2 * kb2 + 2, th * 512:(th + 1) * 512],
                                start=(kb2 == 0), stop=(kb2 == KB2 - 1), perf_mode=DR)
                    for th in range(2):
                        if b1c is not None:
                            bc = b1c(fbk)
                            nc.vector.tensor_scalar(out=pts[th][:], in0=pts[th][:],
                                                    scalar1=bc[:], scalar2=None, op0=ALU.add)
                        nc.scalar.activation(out=g1a[:, fbk, th * 512:(th + 1) * 512],
                                             in_=pts[th][:], func=AF.Gelu_apprx_tanh)
                for tt in range(TT):
                    pws = [ps_mm.tile([P, 384], F32, tag="mm", name="pws%d" % _i) for _i in range(2)]
                    for j in range(FF2):
                        lw = g1a[:, 2 * j:2 * j + 2, tt * P:(tt + 1) * P]
                        for n2 in range(2):
                            nc.tensor.matmul(out=pws[n2][:], lhsT=lw,
                                             rhs=w2t[j][:, :, n2 * 384:(n2 + 1) * 384],
                                             start=(j == 0), stop=(j == FF2 - 1),
                                             perf_mode=DR)
                    for n2 in range(2):
                        sl = h_res[:, tt, n2 * 384:(n2 + 1) * 384]
                        nc.vector.tensor_tensor(out=sl, in0=pws[n2][:], in1=sl, op=ALU.add)
                        if b2f is not None:
                            nc.vector.tensor_tensor(out=sl, in0=sl,
                                                    in1=b2f[:, n2 * 384:(n2 + 1) * 384],
                                                    op=ALU.add)
                # LN2 batched after both th groups so its Ln/Exp (natlog set)
                # doesn't interleave with Gelu ops (gelu set) on ACT.
                ln_all(gb2)
                if ("layer%d" % lay) in tap_d:
                    nc.sync.dma_start(out=tap_d["layer%d" % lay].rearrange("(a p) h -> p a h", p=P),
                                      in_=h_res[:])

            # ---- emissions: logits + log_softmax ----
            hfT = bigp.tile([P, FB, T], BF16, tag="big")
            transpose_to(hfT, lambda tt, fb: h_res[:, tt, fb * P:(fb + 1) * P])
            for tt in range(TT):
                pt = ps_mm.tile([P, L], F32, tag="mm")
                for kb in range(FB):
                    nc.tensor.matmul(out=pt[:], lhsT=hfT[:, kb, tt * P:(tt + 1) * P],
                                     rhs=fcw_sb[:, kb, :], start=(kb == 0), stop=(kb == FB - 1))
                nc.scalar.activation(out=em_sb[:, tt, :], in_=pt[:], func=AF.Copy)
            # batched log-softmax over all TT tiles at once
            mx8 = smp.tile([P, TT], F32, tag="mx8", bufs=1)
            nc.vector.tensor_reduce(out=_xap(mx8[:], [(1, TT)]),
                                    in_=_xap(em_sb[:], [(L, TT), (1, L)]),
                                    axis=AX.X, op=ALU.max)
            nc.vector.tensor_tensor(out=em_sb[:], in0=em_sb[:],
                                    in1=_xap(mx8[:], [(1, TT), (0, L)]), op=ALU.subtract)
            eta = smp.tile([P, TT, L], F32, tag="eta", bufs=1)
            nc.scalar.activation(out=eta[:], in_=em_sb[:], func=AF.Exp)
            sm8 = smp.tile([P, TT], F32, tag="sm8", bufs=1)
            nc.vector.tensor_reduce(out=_xap(sm8[:], [(1, TT)]),
                                    in_=_xap(eta[:], [(L, TT), (1, L)]),
                                    axis=AX.X, op=ALU.add)
            nc.scalar.activation(out=sm8[:], in_=sm8[:], func=AF.Ln)
            nc.vector.tensor_tensor(out=em_sb[:], in0=em_sb[:],
                                    in1=_xap(sm8[:], [(1, TT), (0, L)]), op=ALU.subtract)
            em_d = drp.tile([T, L], F32, tag="em")
            nc.sync.dma_start(out=em_d[:].rearrange("(a p) l -> p a l", p=P), in_=em_sb[:])
            if "em" in tap_d:
                nc.sync.dma_start(out=tap_d["em"].rearrange("(a p) l -> p a l", p=P), in_=em_sb[:])

            # ---- CRF numerator parts (batched across all token tiles) ----
            LL = L * L
            yca = smp.tile([P, TT], F32, tag="yca", bufs=1)
            nc.sync.dma_start(out=yca[:], in_=bass.AP(
                tensor=y_d.tensor, offset=0, ap=[[1, P], [P, TT]]))
            ypa = smp.tile([P, TT], F32, tag="ypa", bufs=1)
            nc.sync.dma_start(out=ypa[1:P, :], in_=bass.AP(
                tensor=y_d.tensor, offset=0, ap=[[1, P - 1], [P, TT]]))
            nc.vector.memset(ypa[0:1, :], 0.0)
            nc.sync.dma_start(out=ypa[0:1, 1:TT], in_=bass.AP(
                tensor=y_d.tensor, offset=P - 1, ap=[[1, 1], [P, TT - 1]]))
            # gold-path mask: zero at each sequence start (p=0, even tile)
            tma = smp.tile([P, TT], F32, tag="tma", bufs=1)
            nc.vector.memset(tma[:], 1.0)
            nc.vector.memset(_xap(tma[0:1], [(2, 4), (0, 1)]), 0.0)
            # start/end bonus rows: start at (p=0, even tt), end at (p=127, odd)
            seb = smp.tile([P, TT, L], F32, tag="seb", bufs=1)
            nc.vector.memset(seb[:], 0.0)
            nc.sync.dma_start(out=_xap(seb[0:1], [(2 * L, 4), (1, L)]), in_=bass.AP(
                tensor=start_d.tensor, offset=0, ap=[[0, 4], [1, L]]))
            nc.sync.dma_start(out=bass.AP(tensor=seb[:].tensor,
                                          offset=seb[:].offset + 127 * TT * L + L,
                                          ap=[[TT * L, 1], [2 * L, 4], [1, L]]),
                              in_=bass.AP(tensor=end_d.tensor, offset=0,
                                          ap=[[0, 1], [0, 4], [1, L]]))
            za = smp.tile([P, TT], F32, tag="za", bufs=1)
            nc.vector.tensor_scalar(out=za[:], in0=ypa[:], scalar1=float(L),
                                    scalar2=None, op0=ALU.mult)
            nc.vector.tensor_tensor(out=za[:], in0=za[:], in1=yca[:], op=ALU.add)
            oh81a = bigp.tile([P, TT * LL], F32, tag="big")
            nc.vector.tensor_tensor(out=_xap(oh81a[:], [(LL, TT), (1, LL)]),
                                    in0=_xap(iota81[:], [(0, TT), (1, LL)]),
                                    in1=_xap(za[:], [(1, TT), (0, LL)]), op=ALU.is_equal)
            nc.vector.tensor_tensor(out=_xap(oh81a[:], [(LL, TT), (1, LL)]),
                                    in0=_xap(oh81a[:], [(LL, TT), (1, LL)]),
                                    in1=_xap(transf[:], [(0, TT), (1, LL)]), op=ALU.mult)
            trsa = smp.tile([P, TT], F32, tag="trsa", bufs=1)
            nc.vector.tensor_reduce(out=_xap(trsa[:], [(1, TT)]),
                                    in_=_xap(oh81a[:], [(LL, TT), (1, LL)]),
                                    axis=AX.X, op=ALU.add)
            nc.vector.tensor_tensor(out=trsa[:], in0=trsa[:], in1=tma[:], op=ALU.mult)
            oh9a = smp.tile([P, TT, L], F32, tag="oh9a", bufs=1)
            nc.vector.tensor_tensor(out=_xap(oh9a[:], [(L, TT), (1, L)]),
                                    in0=_xap(iota9[:], [(0, TT), (1, L)]),
                                    in1=_xap(yca[:], [(1, TT), (0, L)]), op=ALU.is_equal)
            emse = smp.tile([P, TT, L], F32, tag="emse", bufs=1)
            nc.vector.tensor_tensor(out=emse[:], in0=em_sb[:], in1=seb[:], op=ALU.add)
            nc.vector.tensor_tensor(out=emse[:], in0=emse[:], in1=oh9a[:], op=ALU.mult)
            nc.vector.tensor_reduce(out=_xap(part_all[:], [(1, TT)]),
                                    in_=_xap(emse[:], [(L, TT), (1, L)]),
                                    axis=AX.X, op=ALU.add)
            nc.vector.tensor_tensor(out=part_all[:], in0=part_all[:], in1=trsa[:],
                                    op=ALU.add)
            npt = ps_st.tile([1, TT], F32, tag="st")
            nc.tensor.matmul(out=npt[:], lhsT=ones_col[:], rhs=part_all[:], start=True, stop=True)
            nsb = smp.tile([1, TT], F32, tag="nsb")
            nc.scalar.activation(out=nsb[:], in_=npt[:], func=AF.Copy)
            nc.sync.dma_start(out=nums_d[:], in_=nsb[:])

            # ---- CRF denominator: log-semiring reduction tree ----
            # A-side matrices stored flat (i,j); B-side stored TRANSPOSED flat (j,i)
            # so every tensor-op operand AP has <=3 free dims (ISA limit).
            # The tree keeps products packed across partitions so DVE op free
            # sizes shrink with the round (pairs move via SBUF->SBUF DMA).
            L2 = L * L

            def sb_ap(tile_ap, offset, dims):
                return bass.AP(tensor=tile_ap.tensor, offset=tile_ap.offset + offset,
                               ap=[[s, n] for (s, n) in dims])

            def lse_mm_k(A, Bt, Cout, CoutT, pr, k, stable=True):
                """C = A (log-semiring matmul) B, k packed matrices/partition.
                stable=False skips the max-subtraction: legal while entries
                stay well inside exp's f32 range (spans <= 16 steps here)."""
                tmp = bigp.tile([P, BL * L * L * L], F32, tag="big")
                a_ap = _xap(A[0:pr], [(L, k * L), (0, L), (1, L)])
                b_ap = _xap(Bt[0:pr], [(L2, k), (0, L), (1, L2)])
                t3 = _xap(tmp[0:pr], [(L2, k * L), (L, L), (1, L)])
                nc.vector.tensor_tensor(out=t3, in0=a_ap, in1=b_ap, op=ALU.add)
                if stable:
                    mx = cmp_.tile([P, BL, L2], F32, tag="cm")
                    nc.vector.tensor_reduce(out=_xap(mx[0:pr], [(1, k * L2)]),
                                            in_=_xap(tmp[0:pr], [(L, k * L2), (1, L)]),
                                            axis=AX.X, op=ALU.max)
                    mx_b = _xap(mx[0:pr], [(L, k * L), (1, L), (0, L)])
                    nc.vector.tensor_tensor(out=t3, in0=t3, in1=mx_b, op=ALU.subtract)
                nc.scalar.activation(out=_xap(tmp[0:pr], [(1, k * L2 * L)]),
                                     in_=_xap(tmp[0:pr], [(1, k * L2 * L)]), func=AF.Exp)
                sm_ = cmp_.tile([P, BL, L2], F32, tag="cm")
                nc.vector.tensor_reduce(out=_xap(sm_[0:pr], [(1, k * L2)]),
                                        in_=_xap(tmp[0:pr], [(L, k * L2), (1, L)]),
                                        axis=AX.X, op=ALU.add)
                if stable:
                    nc.scalar.activation(out=_xap(sm_[0:pr], [(1, k * L2)]),
                                         in_=_xap(sm_[0:pr], [(1, k * L2)]), func=AF.Ln)
                    nc.vector.tensor_tensor(out=_xap(Cout[0:pr], [(1, k * L2)]),
                                            in0=_xap(sm_[0:pr], [(1, k * L2)]),
                                            in1=_xap(mx[0:pr], [(1, k * L2)]), op=ALU.add)
                    if CoutT is not None:
                        ct_ap = _xap(CoutT[0:pr], [(L2, k), (1, L), (L, L)])
                        s_ap = _xap(sm_[0:pr], [(L2, k), (L, L), (1, L)])
                        m_ap = _xap(mx[0:pr], [(L2, k), (L, L), (1, L)])
                        nc.vector.tensor_tensor(out=ct_ap, in0=s_ap, in1=m_ap,
                                                op=ALU.add)
                else:
                    nc.scalar.activation(out=_xap(Cout[0:pr], [(1, k * L2)]),
                                         in_=_xap(sm_[0:pr], [(1, k * L2)]), func=AF.Ln)
                    if CoutT is not None:
                        nc.scalar.activation(
                            out=_xap(CoutT[0:pr], [(L2, k), (1, L), (L, L)]),
                            in_=_xap(sm_[0:pr], [(L2, k), (L, L), (1, L)]), func=AF.Ln)

            Me = cmp_.tile([P, BL, L2], F32, tag="cm")
            MoT = cmp_.tile([P, BL, L2], F32, tag="cm")
            evod = []
            for par in range(2):
                ev = cmp_.tile([P, BL, L], F32, tag="ev", bufs=4, name="ev")
                for b in range(BL):
                    nc.sync.dma_start(out=ev[:, b, :], in_=bass.AP(
                        tensor=em_d[:].tensor, offset=(b * 256 + par) * L,
                        ap=[[2 * L, P], [1, L]]))
                evod.append(ev)
            nc.vector.tensor_tensor(
                out=Me[:], in0=_xap(transf[:, :], [(0, BL), (1, L2)]),
                in1=_xap(evod[0][:, :, :], [(L, BL), (0, L), (1, L)]), op=ALU.add)
            nc.vector.tensor_tensor(
                out=MoT[:], in0=_xap(transfT[:, :], [(0, BL), (1, L2)]),
                in1=_xap(evod[1][:, :, :], [(L, BL), (1, L), (0, L)]), op=ALU.add)
            nc.sync.dma_start(out=Me[0:1, :, :], in_=bass.AP(
                tensor=pad81_d.tensor, offset=0, ap=[[0, 1], [0, BL], [1, L2]]))

            # R1: 128 products/seq, layout [p=pair, b, 81], k=BL
            Cc = cmp_.tile([P, BL, L2], F32, tag="cm")
            CcT = cmp_.tile([P, BL, L2], F32, tag="cm")
            lse_mm_k(Me, MoT, Cc, CcT, P, BL, stable=False)

            # Shuffles between rounds go through DRAM scratch (partition-
            # crossing strided reads are only supported from DRAM).
            def dram_spill(src, pr, width):
                d = drp.tile([P, BL * L2], F32, tag="cdr", bufs=2)
                nc.sync.dma_start(out=bass.AP(tensor=d[:].tensor, offset=d[:].offset,
                                              ap=[[BL * L2, pr], [1, width]]),
                                  in_=_xap(src[0:pr], [(1, width)]))
                return d

            def dram_row_ap(d, offset, dims):
                return bass.AP(tensor=d[:].tensor, offset=d[:].offset + offset,
                               ap=[[s, n] for (s, n) in dims])

            RW = BL * L2      # dram scratch row width

            # R2: 256 pairs packed [q=(b%2)*64+u, c=b//2]; k=2
            p2 = 2 * L2       # pitch of [P, 2, 81] tiles
            cdr = dram_spill(Cc, P, BL * L2)
            cdrT = dram_spill(CcT, P, BL * L2)
            A2 = cmp_.tile([P, 2, L2], F32, tag="cm")
            Bt2 = cmp_.tile([P, 2, L2], F32, tag="cm")
            for b in range(BL):
                q0 = (b % 2) * 64
                c = b // 2
                nc.sync.dma_start(
                    out=sb_ap(A2[:], q0 * p2 + c * L2, [(p2, 64), (1, L2)]),
                    in_=dram_row_ap(cdr, b * L2, [(2 * RW, 64), (1, L2)]))
                nc.sync.dma_start(
                    out=sb_ap(Bt2[:], q0 * p2 + c * L2, [(p2, 64), (1, L2)]),
                    in_=dram_row_ap(cdrT, RW + b * L2, [(2 * RW, 64), (1, L2)]))
            C2 = cmp_.tile([P, 2, L2], F32, tag="cm")
            C2T = cmp_.tile([P, 2, L2], F32, tag="cm")
            lse_mm_k(A2, Bt2, C2, C2T, P, 2, stable=False)

            # R3: 128 pairs -> [r=b*32+w, 81]; k=1
            cdr = dram_spill(C2, P, 2 * L2)
            cdrT = dram_spill(C2T, P, 2 * L2)
            A3 = cmp_.tile([P, L2], F32, tag="cm")
            Bt3 = cmp_.tile([P, L2], F32, tag="cm")
            for b in range(BL):
                srco = ((b % 2) * 64) * RW + (b // 2) * L2
                nc.sync.dma_start(
                    out=sb_ap(A3[:], (b * 32) * L2, [(L2, 32), (1, L2)]),
                    in_=dram_row_ap(cdr, srco, [(2 * RW, 32), (1, L2)]))
                nc.sync.dma_start(
                    out=sb_ap(Bt3[:], (b * 32) * L2, [(L2, 32), (1, L2)]),
                    in_=dram_row_ap(cdrT, srco + RW, [(2 * RW, 32), (1, L2)]))
            Cr = cmp_.tile([P, L2], F32, tag="cm")
            CrT = cmp_.tile([P, L2], F32, tag="cm")
            lse_mm_k(A3, Bt3, Cr, CrT, P, 1, stable=False)

            # R4..R6: products (b, x) at partition b*npp+x; pair via one
            # strided read per side. Stop at 4 products/seq (span 64).
            for npp, stb in ((32, False), (16, True), (8, True)):
                pr = BL * npp // 2
                cdr = dram_spill(Cr, 2 * pr, L2)
                cdrT = dram_spill(CrT, 2 * pr, L2)
                Ax = cmp_.tile([P, L2], F32, tag="cm")
                Btx = cmp_.tile([P, L2], F32, tag="cm")
                nc.sync.dma_start(out=sb_ap(Ax[:], 0, [(L2, pr), (1, L2)]),
                                  in_=dram_row_ap(cdr, 0, [(2 * RW, pr), (1, L2)]))
                nc.sync.dma_start(out=sb_ap(Btx[:], 0, [(L2, pr), (1, L2)]),
                                  in_=dram_row_ap(cdrT, RW, [(2 * RW, pr), (1, L2)]))
                Cr = cmp_.tile([P, L2], F32, tag="cm")
                CrT = cmp_.tile([P, L2], F32, tag="cm")
                lse_mm_k(Ax, Btx, Cr, CrT, pr, 1, stable=stb)

            # alpha chain over the 4 remaining span-64 factors per sequence,
            # all on partitions 0..3 (one per sequence).
            em0b = cmp_.tile([P, L], F32, tag="ev", bufs=4)
            nc.sync.dma_start(out=em0b[0:BL, :], in_=bass.AP(
                tensor=em_d[:].tensor, offset=0, ap=[[S * L, BL], [1, L]]))
            st4 = cmp_.tile([P, L], F32, tag="ev", bufs=4)
            nc.sync.dma_start(out=st4[0:BL, :], in_=bass.AP(
                tensor=start_d.tensor, offset=0, ap=[[0, BL], [1, L]]))
            en4 = cmp_.tile([P, L], F32, tag="ev", bufs=4)
            nc.sync.dma_start(out=en4[0:BL, :], in_=bass.AP(
                tensor=end_d.tensor, offset=0, ap=[[0, BL], [1, L]]))
            al = cmp_.tile([P, L], F32, tag="ev", bufs=4)
            nc.vector.tensor_tensor(out=al[0:BL, :], in0=em0b[0:BL, :],
                                    in1=st4[0:BL, :], op=ALU.add)
            cdrT = dram_spill(CrT, 4 * BL, L2)
            for z in range(4):
                Pz = cmp_.tile([P, L2], F32, tag="cm")
                nc.sync.dma_start(out=sb_ap(Pz[:], 0, [(L2, BL), (1, L2)]),
                                  in_=dram_row_ap(cdrT, z * RW, [(4 * RW, BL), (1, L2)]))
                t3z = cmp_.tile([P, L2], F32, tag="cm")
                nc.vector.tensor_tensor(out=_xap(t3z[0:BL], [(L, L), (1, L)]),
                                        in0=_xap(al[0:BL], [(0, L), (1, L)]),
                                        in1=_xap(Pz[0:BL], [(L, L), (1, L)]),
                                        op=ALU.add)
                mxz = cmp_.tile([P, L], F32, tag="ev", bufs=4)
                nc.vector.tensor_reduce(out=_xap(mxz[0:BL], [(1, L)]),
                                        in_=_xap(t3z[0:BL], [(L, L), (1, L)]),
                                        axis=AX.X, op=ALU.max)
                nc.vector.tensor_tensor(out=_xap(t3z[0:BL], [(L, L), (1, L)]),
                                        in0=_xap(t3z[0:BL], [(L, L), (1, L)]),
                                        in1=_xap(mxz[0:BL], [(1, L), (0, L)]),
                                        op=ALU.subtract)
                nc.scalar.activation(out=t3z[0:BL, :], in_=t3z[0:BL, :], func=AF.Exp)
                al = cmp_.tile([P, L], F32, tag="ev", bufs=4)
                nc.vector.tensor_reduce(out=_xap(al[0:BL], [(1, L)]),
                                        in_=_xap(t3z[0:BL], [(L, L), (1, L)]),
                                        axis=AX.X, op=ALU.add)
                nc.scalar.activation(out=al[0:BL, :], in_=al[0:BL, :], func=AF.Ln)
                nc.vector.tensor_tensor(out=al[0:BL, :], in0=al[0:BL, :],
                                        in1=mxz[0:BL, :], op=ALU.add)

            # den[b] = lse_j(al[b, j] + end[j])
            nc.vector.tensor_tensor(out=al[0:BL, :], in0=al[0:BL, :],
                                    in1=en4[0:BL, :], op=ALU.add)
            dmx = smp.tile([P, 1], F32, tag="dmx")
            nc.vector.tensor_reduce(out=dmx[0:BL, :], in_=al[0:BL, :], axis=AX.X,
                                    op=ALU.max)
            nc.vector.tensor_scalar(out=al[0:BL, :], in0=al[0:BL, :],
                                    scalar1=dmx[0:BL, :], scalar2=None, op0=ALU.subtract)
            nc.scalar.activation(out=al[0:BL, :], in_=al[0:BL, :], func=AF.Exp)
            dsm = smp.tile([P, 1], F32, tag="dsm")
            nc.vector.tensor_reduce(out=dsm[0:BL, :], in_=al[0:BL, :], axis=AX.X,
                                    op=ALU.add)
            nc.scalar.activation(out=dsm[0:BL, :], in_=dsm[0:BL, :], func=AF.Ln)
            nc.vector.tensor_tensor(out=dsm[0:BL, :], in0=dsm[0:BL, :],
                                    in1=dmx[0:BL, :], op=ALU.add)
            nc.sync.dma_start(
                out=bass.AP(tensor=dens_d.tensor, offset=0, ap=[[1, BL], [1, 1]]),
                in_=dsm[0:BL, :])

    nc.compile()
    return nc


_PROG = {}


def get_program(nl=NL, taps=(), use_bias=False, use_lngb=False):
    key = (nl, tuple(taps), use_bias, use_lngb)
    if key not in _PROG:
        _PROG[key] = build(nl, taps, use_bias, use_lngb)
    return _PROG[key]


def prepare_in_maps(inputs, nl=NL, use_bias=False, use_lngb=False):
    """Shard + preprocess host inputs into per-core in_maps."""
    g = {k: np.asarray(v) for k, v in inputs.items()}
    assert np.all(g["attention_mask"] == 1), "kernel specialized for all-ones attention_mask"
    bf = lambda a: np.ascontiguousarray(a.astype(BF16NP))
    f32 = lambda a: np.ascontiguousarray(a.astype(np.float32))

    def pairw(W):
        """[nl, K, M] -> fp8 [nl, K//256, 128, 2*M]: row r = kb2*256 + c*128 + p."""
        Wn = np.clip(np.asarray(W, np.float32), -240.0, 240.0)
        nlx, Kd, Md = Wn.shape
        Wn = Wn.reshape(nlx, Kd // 256, 2, 128, Md).transpose(0, 1, 3, 2, 4)
        return np.ascontiguousarray(
            Wn.reshape(nlx, Kd // 256, 128, 2 * Md).astype(ml_dtypes.float8_e4m3fn))

    shared = {
        "wemb": bf(g["word_emb"]), "pemb": bf(g["pos_emb"]), "temb": bf(g["type_emb"]),
        "wq": pairw(g["Wq"][:nl]), "wk": pairw(g["Wk"][:nl]), "wv": pairw(g["Wv"][:nl]),
        "wo": pairw(g["Wo"][:nl]), "w1": pairw(g["W1"][:nl]), "w2": pairw(g["W2"][:nl]),
        "fcw": bf(g["fc_w"]),
        "ctrans": f32(g["crf_trans"].reshape(1, L * L)),
        "ctransT": f32(np.ascontiguousarray(np.asarray(g["crf_trans"]).T).reshape(1, L * L)),
        "cstart": f32(g["crf_start"].reshape(1, L)),
        "cend": f32(g["crf_end"].reshape(1, L)),
        "iota9": np.broadcast_to(np.arange(L, dtype=np.float32), (P, L)).copy(),
        "iota81": np.broadcast_to(np.arange(L * L, dtype=np.float32), (P, L * L)).copy(),
        "tmask": np.ascontiguousarray(
            (np.arange(T) % S != 0).astype(np.float32).reshape(T, 1)),
        "pad81": np.where(np.eye(L, dtype=np.float32).reshape(1, L * L) > 0, 0.0,
                          np.float32(NEG)).astype(np.float32),
    }
    if use_bias:
        shared.update({
            "bq": f32(g["bq"][:nl, None, :]), "bk": f32(g["bk"][:nl, None, :]),
            "bv": f32(g["bv"][:nl, None, :]), "bo": f32(g["bo"][:nl, None, :]),
            "b1": f32(g["b1"][:nl, None, :]), "b2": f32(g["b2"][:nl, None, :])})
    if use_lngb:
        lng = np.concatenate([g["emb_ln_g"][None],
                              np.stack([g["ln1_g"], g["ln2_g"]], 1).reshape(2 * nl, H)])[:, None, :]
        lnb = np.concatenate([g["emb_ln_b"][None],
                              np.stack([g["ln1_b"], g["ln2_b"]], 1).reshape(2 * nl, H)])[:, None, :]
        shared.update({"lng": f32(lng), "lnb": f32(lnb)})
    in_maps = []
    for c in range(NCORES):
        sl = slice(c * BL, (c + 1) * BL)
        m = dict(shared)
        m["tok"] = np.ascontiguousarray(g["token_ids"][sl].reshape(T, 1).astype(np.int32))
        m["typ"] = np.ascontiguousarray(g["token_type_ids"][sl].reshape(T, 1).astype(np.int32))
        m["y"] = np.ascontiguousarray(g["y"][sl].reshape(T, 1).astype(np.float32))
        in_maps.append(m)
    return in_maps


def needs_general(inputs):
    g = inputs
    use_bias = any(np.asarray(g[k]).any() for k in ("bq", "bk", "bv", "bo", "b1", "b2"))
    use_lngb = (not np.all(np.asarray(g["ln1_g"]) == 1) or np.asarray(g["ln1_b"]).any()
                or not np.all(np.asarray(g["ln2_g"]) == 1) or np.asarray(g["ln2_b"]).any()
                or not np.all(np.asarray(g["emb_ln_g"]) == 1) or np.asarray(g["emb_ln_b"]).any())
    return use_bias, use_lngb


def loss_from_results(results):
    tot = 0.0
    for r in results:
        tot += float(np.asarray(r["dens"], np.float64).sum()
                     - np.asarray(r["nums"], np.float64).sum())
    return np.float32(tot)


def kernel(**inputs):
    use_bias, use_lngb = needs_general(inputs)
    nc = get_program(NL, (), use_bias, use_lngb)
    in_maps = prepare_in_maps(inputs, NL, use_bias, use_lngb)
    res = run_bass_kernel_spmd(nc, in_maps, list(range(NCORES)))
    return loss_from_results(res.results)

